# revision 7
# baseline (speedup 1.0000x reference)
"""Chamfer distance kernel for 8 Trainium2 NeuronCores.

Problem: template [4, 8192, 3], source [4, 8192, 3] (fp32)
  d[b,n,m] = ||template[b,n] - source[b,m]||^2
  out[b] = mean_n min_m d + mean_m min_n d            (shape [4], fp32)

Sharding: 8 cores = 4 batches x 2 template-halves. Each core computes its
4096x8192 block of the distance matrix ONCE on the TensorEngine (augmented
K=18 matmul: d = n0 + n1 - 2<t,s>, with bf16 hi/lo coordinate splits so
every product is exact in fp32 PSUM accumulation), and reduces it in both
directions:
  - row-mins (over source) via fused tensor_tensor_reduce on DVE
  - col-min partials (over its template half) via a bf16 min-accumulator,
    partition-reduced at the end through PE transposes + DVE reduces.
Host combines: d01 from row-min sums, d10 from elementwise min of the two
halves' col-min vectors.
"""

import numpy as np
import ml_dtypes

BF = ml_dtypes.bfloat16

B = 4
NPTS = 8192  # template points per batch
MPTS = 8192  # source points per batch
NCORES = 8
NT = NPTS // 2  # template rows per core (half batch)
K = 18  # augmented contraction slots
PTILE = 128  # row tile (PSUM partitions)
GW = 2048  # free-dim group width (4 PSUM banks)
NROW = NT // PTILE  # 32 row tiles
NG = MPTS // GW  # 4 column groups
NMM = GW // 512  # matmuls per group
NCOLK = MPTS // PTILE  # 64 columns of colmins output

_BIG = 3.0e38


def _bf16_parts(x64, n):
    """Split float64 array into n bf16 terms; sum of terms ~= x64."""
    parts = []
    r = np.array(x64, dtype=np.float64, copy=True)
    for _ in range(n):
        p = r.astype(BF)
        parts.append(p)
        r -= p.astype(np.float64)
    return parts


def _prep_core(templ_half, source):
    """Build the [K, NT] and [K, MPTS] bf16 slot matrices for one core.

    Slot layout (template side . source side):
      per coord c: (xh, xh, xl, xl) . (-2yh, -2yl, -2yh, -2yl)   -> 12 slots
      n0 (3-way split) . (1, 1, 1)                                -> 3 slots
      (1, 1, 1) . n1 (3-way split)                                -> 3 slots
    so sum_k ta[k,n]*sa[k,m] = ||t~_n - s~_m||^2 (t~, s~ = 16-bit-split
    coordinates; all bf16 products are exact in fp32 accumulation).
    """
    nt = templ_half.shape[0]
    ms = source.shape[0]
    t = templ_half.astype(np.float64)
    s = source.astype(np.float64)
    ta = np.zeros((K, nt), dtype=BF)
    sa = np.zeros((K, ms), dtype=BF)
    t_eff = np.zeros_like(t)
    s_eff = np.zeros_like(s)
    k = 0
    for c in range(3):
        xh, xl = _bf16_parts(t[:, c], 2)
        yh, yl = _bf16_parts(s[:, c], 2)
        t_eff[:, c] = xh.astype(np.float64) + xl.astype(np.float64)
        s_eff[:, c] = yh.astype(np.float64) + yl.astype(np.float64)
        m2yh = (-2.0 * yh.astype(np.float64)).astype(BF)  # exact (x2 = exp+1)
        m2yl = (-2.0 * yl.astype(np.float64)).astype(BF)
        ta[k + 0], sa[k + 0] = xh, m2yh
        ta[k + 1], sa[k + 1] = xh, m2yl
        ta[k + 2], sa[k + 2] = xl, m2yh
        ta[k + 3], sa[k + 3] = xl, m2yl
        k += 4
    n0 = (t_eff**2).sum(axis=1)
    n1 = (s_eff**2).sum(axis=1)
    ones_t = np.ones(nt, dtype=BF)
    ones_s = np.ones(ms, dtype=BF)
    for part in _bf16_parts(n0, 3):
        ta[k], sa[k] = part, ones_s
        k += 1
    for part in _bf16_parts(n1, 3):
        ta[k], sa[k] = ones_t, part
        k += 1
    assert k == K
    return ta, sa


def _build_bass(stage=4):
    from contextlib import ExitStack

    import concourse.bacc as bacc
    import concourse.tile as tile
    from concourse import mybir

    f32 = mybir.dt.float32
    bf16 = mybir.dt.bfloat16
    MIN = mybir.AluOpType.min

    nc = bacc.Bacc("TRN2", target_bir_lowering=False)
    ta = nc.dram_tensor("ta", [K, NT], bf16, kind="ExternalInput")
    sa = nc.dram_tensor("sa", [K, MPTS], bf16, kind="ExternalInput")
    ident = nc.dram_tensor("ident", [PTILE, PTILE], bf16, kind="ExternalInput")
    rowmins = nc.dram_tensor("rowmins", [PTILE, NROW], f32, kind="ExternalOutput")
    colmins = nc.dram_tensor("colmins", [PTILE, NCOLK], f32, kind="ExternalOutput")

    with tile.TileContext(nc) as tc, ExitStack() as ctx:
        consts = ctx.enter_context(tc.tile_pool(name="consts", bufs=1))
        accs = ctx.enter_context(tc.tile_pool(name="accs", bufs=1))
        dpool = ctx.enter_context(tc.tile_pool(name="dpool", bufs=3))
        pspool = ctx.enter_context(tc.tile_pool(name="ps", bufs=2, space="PSUM"))

        ta_s = consts.tile([K, NT], bf16, name="ta_s", tag="ta_s")
        nc.sync.dma_start(out=ta_s, in_=ta[:, :])
        sa_s = consts.tile([K, MPTS], bf16, name="sa_s", tag="sa_s")
        nc.sync.dma_start(out=sa_s, in_=sa[:, :])
        id_s = consts.tile([PTILE, PTILE], bf16, name="id_s", tag="id_s")
        nc.sync.dma_start(out=id_s, in_=ident[:, :])

        colacc = [
            accs.tile([PTILE, GW], bf16, name=f"colacc{g}", tag=f"colacc{g}")
            for g in range(NG)
        ]
        rowacc = accs.tile([PTILE, GW], bf16, name="rowacc", tag="rowacc")
        rowmins_s = accs.tile([PTILE, NROW], f32, name="rowmins_s", tag="rowmins_s")
        colmins_s = accs.tile([PTILE, NCOLK], f32, name="colmins_s", tag="colmins_s")

        for i in range(NROW):
            lhsT = ta_s[:, i * PTILE : (i + 1) * PTILE]
            for g in range(NG):
                ps = pspool.tile([PTILE, GW], f32, name="ps", tag="ps")
                for q in range(NMM):
                    col0 = (g * NMM + q) * 512
                    nc.tensor.matmul(
                        ps[:, q * 512 : (q + 1) * 512],
                        lhsT,
                        sa_s[:, col0 : col0 + 512],
                        start=True,
                        stop=True,
                    )
                d = dpool.tile([PTILE, GW], bf16, name="d", tag="d")
                nc.scalar.copy(d, ps)
                if stage == 5:
                    # Probe: TTR non-in-place
                    dm = dpool.tile([PTILE, GW], bf16, name="dm", tag="dm")
                    init = _BIG if g == 0 else rowmins_s[:, i : i + 1]
                    nc.vector.tensor_tensor_reduce(
                        out=dm,
                        in0=d,
                        in1=d,
                        scale=1.0,
                        scalar=init,
                        op0=MIN,
                        op1=MIN,
                        accum_out=rowmins_s[:, i : i + 1],
                    )
                elif stage >= 2:
                    # Row direction: min-accumulate across the NG groups,
                    # one free-axis reduce per row tile at g == NG-1.
                    if g == 0:
                        nc.vector.tensor_copy(rowacc, d)
                    else:
                        nc.vector.tensor_tensor(out=rowacc, in0=d, in1=rowacc, op=MIN)
                    if g == NG - 1:
                        nc.vector.tensor_reduce(
                            out=rowmins_s[:, i : i + 1],
                            in_=rowacc,
                            axis=mybir.AxisListType.X,
                            op=MIN,
                        )
                else:
                    nc.vector.tensor_reduce(
                        out=rowmins_s[:, i : i + 1],
                        in_=d,
                        axis=mybir.AxisListType.X,
                        op=MIN,
                    )
                # Column direction: elementwise min accumulate.
                if stage >= 3 and stage != 5:
                    if i == 0:
                        nc.vector.tensor_copy(colacc[g], d)
                    else:
                        nc.vector.tensor_tensor(
                            out=colacc[g], in0=d, in1=colacc[g], op=MIN
                        )

        # Partition-reduce the column accumulators: PE transpose 128x128
        # blocks into PSUM (as bf16 slices of the fp32 pool tiles, one per
        # 2KB bank), then DVE free-axis min-reduce each block.
        if stage >= 4:
            kk = 0
            for g in range(NG):
                for t0 in range(0, GW // PTILE, 4):
                    ps = pspool.tile([PTILE, GW], f32, name="ps", tag="ps")
                    psb = ps.bitcast(bf16)  # [128, 2*GW] bf16 view
                    for u in range(4):
                        t = t0 + u
                        nc.tensor.transpose(
                            psb[:, u * 1024 : u * 1024 + PTILE],
                            colacc[g][:, t * PTILE : (t + 1) * PTILE],
                            id_s,
                        )
                    for u in range(4):
                        nc.vector.tensor_reduce(
                            out=colmins_s[:, kk + u : kk + u + 1],
                            in_=psb[:, u * 1024 : u * 1024 + PTILE],
                            axis=mybir.AxisListType.X,
                            op=MIN,
                        )
                    kk += 4
            assert kk == NCOLK
        else:
            nc.vector.memset(colmins_s, 0.0)

        nc.sync.dma_start(out=rowmins[:, :], in_=rowmins_s)
        nc.sync.dma_start(out=colmins[:, :], in_=colmins_s)
    nc.compile()
    return nc


_NC_CACHE = {}


def _get_nc():
    if "nc" not in _NC_CACHE:
        _NC_CACHE["nc"] = _build_bass()
    return _NC_CACHE["nc"]


def kernel(template, source, _trace=False):
    from concourse.bass_utils import run_bass_kernel_spmd

    template = np.asarray(template)
    source = np.asarray(source)
    assert template.shape == (B, NPTS, 3) and source.shape == (B, MPTS, 3)

    eye = np.eye(PTILE, dtype=BF)
    in_maps = []
    for core in range(NCORES):
        b, h = core // 2, core % 2
        ta, sa = _prep_core(template[b, h * NT : (h + 1) * NT], source[b])
        in_maps.append({"ta": ta, "sa": sa, "ident": eye})

    nc = _get_nc()
    res = run_bass_kernel_spmd(
        nc, in_maps, core_ids=list(range(NCORES)), trace=_trace
    )
    results = res.results

    out = np.zeros(B, dtype=np.float64)
    for b in range(B):
        r0, r1 = results[2 * b], results[2 * b + 1]
        d01 = (
            r0["rowmins"].astype(np.float64).sum()
            + r1["rowmins"].astype(np.float64).sum()
        ) / float(NPTS)
        c0 = r0["colmins"].T.reshape(-1)  # [MPTS], source idx = 128*k + p
        c1 = r1["colmins"].T.reshape(-1)
        d10 = np.minimum(c0, c1).astype(np.float64).mean()
        out[b] = d01 + d10
    if _trace:
        kernel._last_results = res
    return out.astype(np.float32)


# revision 10
# speedup vs baseline: 1.0648x; 1.0648x over previous
"""Chamfer distance kernel for 8 Trainium2 NeuronCores.

Problem: template [4, 8192, 3], source [4, 8192, 3] (fp32)
  d[b,n,m] = ||template[b,n] - source[b,m]||^2
  out[b] = mean_n min_m d + mean_m min_n d            (shape [4], fp32)

Sharding: 8 cores = 4 batches x 2 template-halves. Each core computes its
4096x8192 block of the distance matrix ONCE on the TensorEngine (augmented
K=18 matmul: d = n0 + n1 - 2<t,s>, with bf16 hi/lo coordinate splits so
every product is exact in fp32 PSUM accumulation), and reduces it in both
directions:
  - ScalarE converts each PSUM tile to a bf16 SBUF row-panel (it is the
    only engine that can read PSUM while DVE does the min work).
  - col-min partials: DVE tensor_tensor min accumulators (bf16 2x mode),
    partition-reduced at the end through PE transposes + DVE reduces.
  - row-mins: in-place log2 halving chain of TT-mins on the row panel,
    then one small reduce.
Host combines: d01 from row-min sums, d10 from elementwise min of the two
halves' col-min vectors.
"""

import numpy as np
import ml_dtypes

BF = ml_dtypes.bfloat16

B = 4
NPTS = 8192  # template points per batch
MPTS = 8192  # source points per batch
NCORES = 8
NT = NPTS // 2  # template rows per core (half batch)
K = 18  # augmented contraction slots
PTILE = 128  # row tile (PSUM partitions)
CW = 2048  # ScalarE copy width (4 PSUM banks per psum tile)
NCP = MPTS // CW  # 2 copies per row tile
GW = 2048  # colacc accumulator width
NG = MPTS // GW  # 4 column groups
NROW = NT // PTILE  # 32 row tiles
NCOLK = MPTS // PTILE  # 64 columns of colmins output
HALVE_STOP = 256  # stop the halving chain here, reduce the rest

_BIG = 3.0e38


def _bf16_parts(x64, n):
    """Split float64 array into n bf16 terms; sum of terms ~= x64."""
    parts = []
    r = np.array(x64, dtype=np.float64, copy=True)
    for _ in range(n):
        p = r.astype(BF)
        parts.append(p)
        r -= p.astype(np.float64)
    return parts


def _prep_core(templ_half, source):
    """Build the [K, NT] and [K, MPTS] bf16 slot matrices for one core.

    Slot layout (template side . source side):
      per coord c: (xh, xh, xl, xl) . (-2yh, -2yl, -2yh, -2yl)   -> 12 slots
      n0 (3-way split) . (1, 1, 1)                                -> 3 slots
      (1, 1, 1) . n1 (3-way split)                                -> 3 slots
    so sum_k ta[k,n]*sa[k,m] = ||t~_n - s~_m||^2 (t~, s~ = 16-bit-split
    coordinates; all bf16 products are exact in fp32 accumulation).
    """
    nt = templ_half.shape[0]
    ms = source.shape[0]
    t = templ_half.astype(np.float64)
    s = source.astype(np.float64)
    ta = np.zeros((K, nt), dtype=BF)
    sa = np.zeros((K, ms), dtype=BF)
    t_eff = np.zeros_like(t)
    s_eff = np.zeros_like(s)
    k = 0
    for c in range(3):
        xh, xl = _bf16_parts(t[:, c], 2)
        yh, yl = _bf16_parts(s[:, c], 2)
        t_eff[:, c] = xh.astype(np.float64) + xl.astype(np.float64)
        s_eff[:, c] = yh.astype(np.float64) + yl.astype(np.float64)
        m2yh = (-2.0 * yh.astype(np.float64)).astype(BF)  # exact (x2 = exp+1)
        m2yl = (-2.0 * yl.astype(np.float64)).astype(BF)
        ta[k + 0], sa[k + 0] = xh, m2yh
        ta[k + 1], sa[k + 1] = xh, m2yl
        ta[k + 2], sa[k + 2] = xl, m2yh
        ta[k + 3], sa[k + 3] = xl, m2yl
        k += 4
    n0 = (t_eff**2).sum(axis=1)
    n1 = (s_eff**2).sum(axis=1)
    ones_t = np.ones(nt, dtype=BF)
    ones_s = np.ones(ms, dtype=BF)
    for part in _bf16_parts(n0, 3):
        ta[k], sa[k] = part, ones_s
        k += 1
    for part in _bf16_parts(n1, 3):
        ta[k], sa[k] = ones_t, part
        k += 1
    assert k == K
    return ta, sa


def _build_bass(gpsimd_frac=0):
    """gpsimd_frac: out of 4, how many row-tiles per 4 have their colacc
    updates run on GpSimd instead of DVE (load balancing experiment)."""
    from contextlib import ExitStack

    import concourse.bacc as bacc
    import concourse.tile as tile
    from concourse import mybir

    f32 = mybir.dt.float32
    bf16 = mybir.dt.bfloat16
    MIN = mybir.AluOpType.min

    nc = bacc.Bacc("TRN2", target_bir_lowering=False)
    ta = nc.dram_tensor("ta", [K, NT], bf16, kind="ExternalInput")
    sa = nc.dram_tensor("sa", [K, MPTS], bf16, kind="ExternalInput")
    ident = nc.dram_tensor("ident", [PTILE, PTILE], bf16, kind="ExternalInput")
    rowmins = nc.dram_tensor("rowmins", [PTILE, NROW], f32, kind="ExternalOutput")
    colmins = nc.dram_tensor("colmins", [PTILE, NCOLK], f32, kind="ExternalOutput")

    with tile.TileContext(nc) as tc, ExitStack() as ctx:
        consts = ctx.enter_context(tc.tile_pool(name="consts", bufs=1))
        accs = ctx.enter_context(tc.tile_pool(name="accs", bufs=1))
        dpool = ctx.enter_context(tc.tile_pool(name="dpool", bufs=2))
        pspool = ctx.enter_context(tc.tile_pool(name="ps", bufs=2, space="PSUM"))

        ta_s = consts.tile([K, NT], bf16, name="ta_s", tag="ta_s")
        nc.sync.dma_start(out=ta_s, in_=ta[:, :])
        sa_s = consts.tile([K, MPTS], bf16, name="sa_s", tag="sa_s")
        nc.sync.dma_start(out=sa_s, in_=sa[:, :])
        id_s = consts.tile([PTILE, PTILE], bf16, name="id_s", tag="id_s")
        nc.sync.dma_start(out=id_s, in_=ident[:, :])

        colacc = [
            accs.tile([PTILE, GW], bf16, name=f"colacc{g}", tag=f"colacc{g}")
            for g in range(NG)
        ]
        rowmins_s = accs.tile([PTILE, NROW], f32, name="rowmins_s", tag="rowmins_s")
        colmins_s = accs.tile([PTILE, NCOLK], f32, name="colmins_s", tag="colmins_s")

        for i in range(NROW):
            lhsT = ta_s[:, i * PTILE : (i + 1) * PTILE]
            d = dpool.tile([PTILE, MPTS], bf16, name="d", tag="d")
            for cp in range(NCP):
                ps = pspool.tile([PTILE, CW], f32, name="ps", tag="ps")
                for q in range(CW // 512):
                    col0 = cp * CW + q * 512
                    nc.tensor.matmul(
                        ps[:, q * 512 : (q + 1) * 512],
                        lhsT,
                        sa_s[:, col0 : col0 + 512],
                        start=True,
                        stop=True,
                    )
                nc.scalar.copy(d[:, cp * CW : (cp + 1) * CW], ps)
            # Column direction: elementwise min accumulate per 2048-group.
            eng = nc.gpsimd if (i % 4) < gpsimd_frac and i > 0 else nc.vector
            for g in range(NG):
                dg = d[:, g * GW : (g + 1) * GW]
                if i == 0:
                    nc.vector.tensor_copy(colacc[g], dg)
                else:
                    eng.tensor_tensor(out=colacc[g], in0=dg, in1=colacc[g], op=MIN)
            # Row direction: in-place halving chain, then one small reduce.
            w = MPTS // 2
            while w >= HALVE_STOP:
                nc.vector.tensor_tensor(
                    out=d[:, :w], in0=d[:, :w], in1=d[:, w : 2 * w], op=MIN
                )
                w //= 2
            nc.vector.tensor_reduce(
                out=rowmins_s[:, i : i + 1],
                in_=d[:, : 2 * w],
                axis=mybir.AxisListType.X,
                op=MIN,
            )

        # Partition-reduce the column accumulators: PE transpose 128x128
        # blocks into PSUM (as bf16 slices of the fp32 pool tiles, one per
        # 2KB bank), then DVE segmented min-reduce (3D AP, axis X).
        kk = 0
        nper = CW // 512  # transposes per psum tile (one per bank)
        for t0 in range(0, NCOLK, nper):
            ps = pspool.tile([PTILE, CW], f32, name="ps", tag="ps")
            psb = ps.bitcast(bf16)  # [128, 2*CW] bf16 view
            for u in range(nper):
                t = t0 + u  # source block index: points 128*t .. 128*t+127
                g, tin = t // (GW // PTILE), t % (GW // PTILE)
                nc.tensor.transpose(
                    psb[:, u * 1024 : u * 1024 + PTILE],
                    colacc[g][:, tin * PTILE : (tin + 1) * PTILE],
                    id_s,
                )
            seg = psb.rearrange("p (n x) -> p n x", x=1024)[:, :, :PTILE]
            nc.vector.tensor_reduce(
                out=colmins_s[:, kk : kk + nper],
                in_=seg,
                axis=mybir.AxisListType.X,
                op=MIN,
            )
            kk += nper
        assert kk == NCOLK

        nc.sync.dma_start(out=rowmins[:, :], in_=rowmins_s)
        nc.sync.dma_start(out=colmins[:, :], in_=colmins_s)
    nc.compile()
    return nc


_NC_CACHE = {}


def _get_nc():
    if "nc" not in _NC_CACHE:
        _NC_CACHE["nc"] = _build_bass()
    return _NC_CACHE["nc"]


def kernel(template, source, _trace=False):
    from concourse.bass_utils import run_bass_kernel_spmd

    template = np.asarray(template)
    source = np.asarray(source)
    assert template.shape == (B, NPTS, 3) and source.shape == (B, MPTS, 3)

    eye = np.eye(PTILE, dtype=BF)
    in_maps = []
    for core in range(NCORES):
        b, h = core // 2, core % 2
        ta, sa = _prep_core(template[b, h * NT : (h + 1) * NT], source[b])
        in_maps.append({"ta": ta, "sa": sa, "ident": eye})

    nc = _get_nc()
    res = run_bass_kernel_spmd(
        nc, in_maps, core_ids=list(range(NCORES)), trace=_trace
    )
    results = res.results

    out = np.zeros(B, dtype=np.float64)
    for b in range(B):
        r0, r1 = results[2 * b], results[2 * b + 1]
        d01 = (
            r0["rowmins"].astype(np.float64).sum()
            + r1["rowmins"].astype(np.float64).sum()
        ) / float(NPTS)
        c0 = r0["colmins"].T.reshape(-1)  # [MPTS], source idx = 128*k + p
        c1 = r1["colmins"].T.reshape(-1)
        d10 = np.minimum(c0, c1).astype(np.float64).mean()
        out[b] = d01 + d10
    if _trace:
        kernel._last_results = res
    return out.astype(np.float32)
